# revision 35
# baseline (speedup 1.0000x reference)
"""Trainium2 Bass kernel for nn_CMR_59931973648949 (gnn_message_passing).

Contract: kernel(**inputs) takes FULL unsharded numpy inputs and returns the
FULL [16, 1024] output. Data-parallel over batch across 8 cores (2 samples
per core, weights replicated). All weights are host-packed partition-major
([128, F], fp16) for max-bandwidth DMAs; the two local samples are batched
through the shared heavy matmuls (feat_v, q/u0/u1 projections). The ~9.8MB
weight stream (dominated by W_out 4MB + W_v 2MB fp16) is the kernel floor;
everything else is arranged to hide under it:

- Readout restructure: YT[n, dc] = sum_v visf[v, n]*WoT[v, dc] accumulates
  chunk-by-chunk as the WoT stream lands (independent of the chain result);
  the tail is outT[p, j, s] = YT^T-slices @ fa2 in a [128, (j, s)] layout so
  every epilogue op runs on 128 partitions. The last two WoT v-chunks
  (12..15) skip YT and enter outT directly via w_c = visf_c^T @ fa2, so no
  PSUM->SBUF copy of YT trails the last DMA byte.
- Act table discipline: the engine holds one resident set and a reload is
  ~1.3us, so the edge sigmoid is computed as 0.5*tanh(x/2)+0.5 (tanh shares
  the Exp set). The affine half folds into gfT (x0.5) and a k=1 crow matmul;
  a dummy Exp anchored after the sqrt pins the load order sqrt->exp with no
  mid-chain switches.
- The final attention peak-norm is a GPSIMD partition-axis (C) reduce on the
  SBUF fa column (no PE transpose), and the 1/nr scale is applied in the
  epilogue via a PE ones-broadcast (nrbc), keeping the chain short.

Math per sample (see reference):
  scl[n] = mean(norm_w)/max(||visf[:,n]||,1e-12)   (folded into feat_v scale)
  feat_v = (visf.T * scl) @ W_v.T ; used only via feat_vT
  q/u0/u1 from node/relate reps with WnT=W_node.T/sqrt(DV),
      WA0/1=W_rel.T@W_e[:, :DV | DV:]/sqrt(DE)
  find = softmax(mask(q @ feat_vT)) * node_mask
  ea_r = sigmoid(A0[r,:] bcast + A1T[:,r]) * relation_mask
  g_findT = find.T-gather via GT (folds valid*relate_mask*onehot(obj))
  h[r,:] = g_find[r,:] @ ea_r ; find2T = findT + h.T @ ST (onehot(subj))
  fa = rowmax(find2T); fa /= max(max(fa),1); fa = fa*bm + (1-bm)*1e-7
  out[s, 128j+p] = outT[p, j, s] = (fa2^T YT + w^T WoT)*nr + b_out
"""

import numpy as np

import concourse.bass as bass
from concourse import bass_isa
import concourse.tile as tile
from concourse import bacc, mybir
from concourse.bass_utils import run_bass_kernel_spmd

P = 128
B, K, R, N = 16, 12, 12, 64
DW, DV, DVIS, DE, DC = 512, 512, 2048, 512, 1024
NCORES = 8
S = B // NCORES  # samples per core = 2
N2 = S * N  # 128: both samples' boxes side by side
K2 = S * K  # 24

F32 = mybir.dt.float32
F32R = mybir.dt.float32r
BF16 = mybir.dt.bfloat16
F16 = mybir.dt.float16
E3 = mybir.dt.float8e3  # fp8 e3m4: 4-bit mantissa, max 15.5
USE_F32R = True
HALF = F16        # half dtype for weight DMAs (fp16: 10-bit mantissa)
WVT_BF16 = True   # feat_v matmul operands in half
WCAT_BF16 = True  # q/u0/u1 weight + reps in bf16
WOT_BF16 = True   # W_out matmul operands in half (fp16 keeps ~3e-4)
import ml_dtypes

E3NP = ml_dtypes.float8_e3m4
E3_TARGET = 8.0   # quantized absmax target (<= 15.5 with 2x headroom)


def _pow2_scale(a):
    """Power-of-2 scale s s.t. absmax(a*s) lands just under E3_TARGET."""
    amax = float(np.abs(a).max())
    return float(2.0 ** np.floor(np.log2(E3_TARGET / max(amax, 1e-30))))

# smalls packing column offsets (per sample, [64, SMALLS_F])
_SM_RM = 0          # rmask      [64, 64]
_SM_BM = 64         # bmmul      [12, 64]
_SM_BA = 128        # bmadd      [12, 64]
_SM_GT = 192        # GT         [12, 12]
_SM_ST = 204        # ST         [12, 12]
_SM_NM = 216        # nmcol      [12, 1]
_SM_FM = 217        # famul row  [1, 64]
_SM_FA = 281        # faadd row  [1, 64]
SMALLS_F = 345

_cache = {}


def _pack(a):
    """[(o*128), F] row-major -> [128, o*F] partition-major."""
    o = a.shape[0] // P
    return np.ascontiguousarray(
        a.reshape(o, P, a.shape[1]).transpose(1, 0, 2).reshape(P, -1)
    )


def build_nc(bm_ones=False, nm_ones=False, rm_ones=False,
             scales=(1.0, 1.0, 1.0, 1.0, 1.0)):
    # scales = (s_wv, s_wn, s_a0, s_a1): power-of-2 fp8 quantization scales
    # baked into the weight streams; de-scaled at the existing PSUM->SBUF
    # copies (q/u0/u1 pre-compensate feat_v's s_wv since logits/A0/A1 are
    # the only consumers of feat_v).
    s_wv, s_wn, s_a0, s_a1, s_wo = scales
    nc = bacc.Bacc(num_devices=NCORES)

    FR = F32R if USE_F32R else F32
    d_visf = nc.declare_dram_parameter("visf16", [P, 16 * N2], HALF, isOutput=False)
    d_WvT = nc.declare_dram_parameter("WvT", [P, 16 * DV], E3, isOutput=False)
    # wcat = nrepT2 | rrepT2 | I12 (fp16); wq = WnT | WA0 | WA1 (fp8 e3m4)
    WCATF = 2 * 4 * K2 + K
    d_wcat = nc.declare_dram_parameter("wcat", [P, WCATF], HALF, isOutput=False)
    d_wq = nc.declare_dram_parameter("wq", [P, 3 * 4 * DV], E3, isOutput=False)
    d_WoT = nc.declare_dram_parameter("WoT", [P, 16 * DC], E3, isOutput=False)
    # bias transposed to the outT layout: boutT[p, (j, s)] = b_out[128j+p]
    d_bout = nc.declare_dram_parameter("bout", [P, 16], HALF, isOutput=False)
    # resth = I128 [P, 128] | smalls [64, S*smf]  (all fp16). In the
    # all-ones fast path only GT/ST are consumed, so the smalls shrink from
    # 345 to 24 columns per sample (~82KB less stream traffic).
    fast = bm_ones and nm_ones and rm_ones
    # fast-path smalls: GS [K, R*K] at 0, t row [1, K] at R*K
    smf = (R * K + K) if fast else SMALLS_F
    d_rest = nc.declare_dram_parameter(
        "resth", [P, P + S * smf], HALF, isOutput=False
    )
    # transposed output: d_out[p, 2j+s] = out[s, 128j+p]; host un-permutes
    d_out = nc.declare_dram_parameter("out", [P, 16], F32, isOutput=True)

    with tile.TileContext(nc) as tc:
        with (
            tc.tile_pool(name="singles", bufs=1) as singles,
            tc.tile_pool(name="ps", bufs=2) as ps,
            tc.tile_pool(name="psum", bufs=5, space="PSUM") as psum,
            tc.tile_pool(name="psumT", bufs=2, space="PSUM") as psumT,
            tc.tile_pool(name="psumO", bufs=1, space="PSUM") as psumO,
        ):
            # ---- DMAs on the critical path first (SP queue runs in order) ----
            visf2_mm = singles.tile([P, 16, N2], HALF)
            nc.sync.dma_start(
                out=visf2_mm[:], in_=d_visf[:].rearrange("p (o n) -> p o n", o=16)
            )
            # I128 gates the scl chain and every PE transpose — land it first
            rest_sb = singles.tile([P, P + S * smf], HALF)
            nc.sync.dma_start(out=rest_sb[:, :P], in_=d_rest[:, :P])
            # reps+WnT land before the big W_v stream: the q/u0/u1 gate is
            # the chain-start bottleneck, feat_v is not
            wcatall_sb = singles.tile([P, WCATF], HALF)
            nc.sync.dma_start(out=wcatall_sb[:], in_=d_wcat[:])
            WvT_sb = singles.tile([P, 16, DV], E3)
            for g in range(2):
                nc.sync.dma_start(
                    out=WvT_sb[:, 8 * g : 8 * g + 8, :],
                    in_=d_WvT[:, 8 * g * DV : 8 * (g + 1) * DV].rearrange(
                        "p (o d) -> p o d", o=8
                    ),
                )
            wq_sb = singles.tile([P, 3, 4, DV], E3)
            nc.sync.dma_start(
                out=wq_sb[:, 0],
                in_=d_wq[:, : 4 * DV].rearrange("p (o d) -> p o d", o=4),
            )
            nc.sync.dma_start(
                out=wq_sb[:, 1:],
                in_=d_wq[:, 4 * DV :].rearrange("p (t o d) -> p t o d", t=2, o=4),
            )
            # W_out weight stream: 7 transfers of 2 v-chunks, then c14 and
            # c15 alone — the final two chunks feed the outT accumulation
            # directly (w-trick) so no YT copy trails the stream. The smalls
            # (masks/GT/ST) and bias slot in after the first WoT group: they
            # are not needed until the proj stage (~17us), and keeping them
            # out of the front moves the last WoT byte ~0.5us earlier.
            WoT_sb = singles.tile([P, 16, DC], E3)
            boutT_sb = singles.tile([P, 8, S], HALF)
            for g in range(7):
                nc.sync.dma_start(
                    out=WoT_sb[:, 2 * g : 2 * g + 2, :],
                    in_=d_WoT[:, 2 * g * DC : 2 * (g + 1) * DC].rearrange(
                        "p (o d) -> p o d", o=2
                    ),
                )
                if g == 0:
                    nc.sync.dma_start(out=rest_sb[:, P:], in_=d_rest[:, P:])
                    nc.sync.dma_start(
                        out=boutT_sb[:],
                        in_=d_bout[:].rearrange("p (j s) -> p j s", j=8),
                    )
            for c in (14, 15):
                nc.sync.dma_start(
                    out=WoT_sb[:, c : c + 1, :],
                    in_=d_WoT[:, c * DC : (c + 1) * DC].rearrange(
                        "p (o d) -> p o d", o=1
                    ),
                )

            I128_sb = rest_sb[:, :P]
            smalls_sb = [
                rest_sb[:N, P + s * smf : P + (s + 1) * smf]
                for s in range(S)
            ]
            reps_sb = wcatall_sb[:, : 2 * 4 * K2].rearrange(
                "p (t o k) -> p t o k", t=2, o=4
            )
            I12h_sb = wcatall_sb[:K, 2 * 4 * K2 : 2 * 4 * K2 + K]
            WnT_sb = wq_sb[:, 0]
            WA0_sb = wq_sb[:, 1]
            WA1_sb = wq_sb[:, 2]

            nrep2 = reps_sb[:, 0]  # [P, 4, 24]
            rrep2 = reps_sb[:, 1]
            HALF_SM = HALF if WCAT_BF16 else F32  # dtype of small attention mms

            # fa2[:, s] holds sample s's (normalized, masked) final
            # attention column; zero outside its 64-row block
            fa2_sb = singles.tile([N2, S], HALF)
            nc.gpsimd.memset(fa2_sb[:], 0.0)
            ones1_sb = singles.tile([1, S], HALF)
            nc.gpsimd.memset(ones1_sb[:], 1.0)
            # peak-norm reciprocals as a [1, S] row; broadcast across the
            # 128 partitions via a PE ones-outer-product for the epilogue
            nr2row_sb = singles.tile([1, S], F32)
            ones128r_sb = singles.tile([1, P], F32)
            nc.gpsimd.memset(ones128r_sb[:], 1.0)
            ones128c_sb = singles.tile([P, 1], HALF)
            nc.gpsimd.memset(ones128c_sb[:], 1.0)
            ones128rh_sb = singles.tile([1, P], HALF)
            nc.gpsimd.memset(ones128rh_sb[:], 1.0)

            # The readout is a pure w-trick: w_c = visfT_c @ fa2 is the
            # mem chunk, and outT accumulates WoT_c^T @ w_c as soon as fa2
            # is ready — no YT intermediate, nothing big trails the stream.
            # The 16 PE transposes also double as the PE p-state warmup (the
            # cost model needs ~3us of continuous PE activity before full
            # rate); copies go DVE-only so the Act table stays on sqrt->exp,
            # and a dedicated PSUM pool keeps featv_ps allocation unblocked.
            W_CHUNKS = tuple(range(16))
            visfT_sb = []
            for c in W_CHUNKS:
                vT_ps = psumT.tile([N2, P], F32, tag="vt", name=f"vT_ps{c}")
                nc.tensor.matmul(
                    out=vT_ps[:], lhsT=visf2_mm[:, c, :], rhs=I128_sb[:],
                    start=True, stop=True,
                )
                vT = singles.tile([N2, P], HALF, name=f"vT{c}")
                nc.vector.tensor_copy(out=vT[:], in_=vT_ps[:])
                visfT_sb.append(vT)

            # ---- column norms: elementwise square (DVE) then per-chunk
            # ones-column matmuls accumulate the partition sums — only the
            # diagonal of the old gram product, at ~1/4 the PE time ----
            sq_sb = singles.tile([P, 16, N2], HALF)
            nc.vector.tensor_tensor(
                out=sq_sb[:], in0=visf2_mm[:], in1=visf2_mm[:],
                op=mybir.AluOpType.mult,
            )
            nrm2_ps = psum.tile([N2, 1], F32, tag="ps", name="nrm2")
            for c in range(16):
                nc.tensor.matmul(
                    out=nrm2_ps[:],
                    lhsT=sq_sb[:, c, :],
                    rhs=ones128c_sb[:],
                    start=(c == 0),
                    stop=(c == 15),
                )
            scl = singles.tile([N2, 1], F32)
            nc.scalar.sqrt(out=scl[:], in_=nrm2_ps[:])
            nc.vector.tensor_scalar_max(out=scl[:], in0=scl[:], scalar1=1e-12)
            nc.vector.reciprocal(out=scl[:], in_=scl[:])
            # prefetch the Exp/Tanh table right after the sqrt: the Act table
            # slot is single; sqrt->exp is the only transition and both
            # loads hide under the weight stream. The edge nonlinearity uses
            # tanh (same set) — sigmoid's set is never touched.
            dummy_sb = singles.tile([1, 1], F32)
            nc.scalar.activation(
                out=dummy_sb[:], in_=scl[:1, :],
                func=mybir.ActivationFunctionType.Exp,
            )

            # ---- shared: qT/u0T/u1T for both samples [d, 24] ----
            def lin_T(w_sb, x_ap, name, unscale, eng="act"):
                out_ps = psum.tile([P, 4, K2], F32, tag="ps", name=name + "_ps")
                for dc in range(4):
                    for wc in range(4):
                        nc.tensor.matmul(
                            out=out_ps[:, dc, :],
                            lhsT=w_sb[:, wc, P * dc : P * (dc + 1)],
                            rhs=x_ap[:, wc, :],
                            start=(dc == 0 and wc == 0),
                            stop=(dc == 3 and wc == 3),
                        )
                out_sb = singles.tile([P, 4, K2], HALF_SM, name=name)
                # the copy doubles as fp8 de-quantization (own scale and
                # feat_v's s_wv, since the product q.feat_v must be exact)
                if eng == "act":
                    nc.scalar.mul(out=out_sb[:], in_=out_ps[:], mul=unscale)
                else:
                    nc.vector.tensor_scalar_mul(
                        out=out_sb[:], in0=out_ps[:], scalar1=unscale
                    )
                return out_sb

            # ---- shared: feat_v for both samples [n2, 512], blocked by
            # 128-column output group so each group's scale-copy (Act/DVE
            # alternating), transpose, and ftT2 copy pipeline behind the
            # matmuls instead of waiting for the full 512 columns ----
            featv_ps = psum.tile([N2, DV], F32, tag="ps")
            for dc in range(4):
                for c in range(16):
                    nc.tensor.matmul(
                        out=featv_ps[:, P * dc : P * (dc + 1)],
                        lhsT=visf2_mm[:, c, :],
                        rhs=WvT_sb[:, c, P * dc : P * (dc + 1)],
                        start=(c == 0),
                        stop=(c == 15),
                    )
            featv_sb = singles.tile([N2, DV], HALF)
            ftT2_ps = psum.tile([P, 4, N2], F32, tag="ps")
            ftT2_sb = singles.tile([P, 4, N2], HALF_SM)
            for dc in range(4):
                blk = slice(P * dc, P * (dc + 1))
                if dc % 2 == 0:
                    nc.scalar.mul(
                        out=featv_sb[:, blk], in_=featv_ps[:, blk], mul=scl[:]
                    )
                else:
                    nc.vector.tensor_scalar_mul(
                        out=featv_sb[:, blk], in0=featv_ps[:, blk],
                        scalar1=scl[:],
                    )
                nc.tensor.matmul(
                    out=ftT2_ps[:, dc, :], lhsT=featv_sb[:, blk],
                    rhs=I128_sb[:], start=True, stop=True,
                )
                if dc % 2 == 0:
                    nc.vector.tensor_copy(
                        out=ftT2_sb[:, dc], in_=ftT2_ps[:, dc]
                    )
                else:
                    nc.scalar.copy(out=ftT2_sb[:, dc], in_=ftT2_ps[:, dc])
            # q/u0/u1 projections follow on PE (their wq data lands
            # mid-featv; the consumers A0/A1/logits come later anyway)
            qT2_sb = lin_T(WnT_sb, nrep2, "qT2", 1.0 / (s_wn * s_wv))
            u0T2_sb = lin_T(WA0_sb, rrep2, "u0T2", 1.0 / (s_a0 * s_wv), "dve")
            u1T2_sb = lin_T(WA1_sb, rrep2, "u1T2", 1.0 / (s_a1 * s_wv))

            # ---- per-sample pipeline, stages interleaved across samples ----
            st = [dict() for _ in range(S)]
            for s in range(S):
                sm = smalls_sb[s]
                if fast:
                    st[s]["GSm"] = sm[:K, 0 : R * K]
                    st[s]["tm"] = sm[:1, R * K : R * K + K]
                else:
                    st[s]["GTm"] = sm[:K, _SM_GT : _SM_GT + R]
                    st[s]["STm"] = sm[:R, _SM_ST : _SM_ST + K]
                if not fast:
                    st[s]["rmask"] = sm[:, _SM_RM : _SM_RM + N]
                    st[s]["bmmul"] = sm[:K, _SM_BM : _SM_BM + N]
                    st[s]["bmadd"] = sm[:K, _SM_BA : _SM_BA + N]
                    st[s]["nmcol"] = sm[:K, _SM_NM : _SM_NM + 1]
                    st[s]["famul"] = sm[:1, _SM_FM : _SM_FM + N]
                    st[s]["faadd"] = sm[:1, _SM_FA : _SM_FA + N]
                st[s]["ks"] = slice(K * s, K * (s + 1))
                st[s]["ns"] = slice(N * s, N * (s + 1))

            # ---- shared: A0/A1 for both samples, batched [R, S, N] (they
            # depend only on u0/u1 + ftT2, not on find — so the edge Bg and
            # the Act-serial sigmoid chain can start before the softmax) ----
            A0_ps = psum.tile([R, S, N], F32, tag="ps", name="A0_ps")
            A1_ps = psum.tile([R, S, N], F32, tag="ps", name="A1_ps")
            for u_sb, ps_t in ((u0T2_sb, A0_ps), (u1T2_sb, A1_ps)):
                for s in range(S):
                    d = st[s]
                    for c in range(4):
                        nc.tensor.matmul(
                            out=ps_t[:, s, :], lhsT=u_sb[:, c, d["ks"]],
                            rhs=ftT2_sb[:, c, d["ns"]],
                            start=(c == 0), stop=(c == 3),
                        )
            A0_sb = singles.tile([R, S, N], HALF_SM, name="A0")
            nc.scalar.copy(out=A0_sb[:], in_=A0_ps[:])
            A1_sb = singles.tile([R, S, N], HALF_SM, name="A1")
            nc.vector.tensor_copy(out=A1_sb[:], in_=A1_ps[:])
            for s in range(S):
                st[s]["A0"] = A0_sb[:, s, :]
                st[s]["A1"] = A1_sb[:, s, :]

            def softmax2():
                # fast path: both samples in one pass. Logits are O(1), so
                # exp runs without the max-subtract (mathematically equal)
                lg2_ps = psum.tile([K, S, N], F32, tag="ps", name="lg2")
                for s in range(S):
                    d = st[s]
                    for c in range(4):
                        nc.tensor.matmul(
                            out=lg2_ps[:, s, :], lhsT=qT2_sb[:, c, d["ks"]],
                            rhs=ftT2_sb[:, c, d["ns"]],
                            start=(c == 0), stop=(c == 3),
                        )
                e2_sb = singles.tile([K, S, N], F32, name="e2")
                nc.scalar.activation(
                    out=e2_sb[:], in_=lg2_ps[:],
                    func=mybir.ActivationFunctionType.Exp,
                )
                ss2 = singles.tile([K, S, 1], F32, name="ss2")
                nc.vector.tensor_reduce(
                    out=ss2[:], in_=e2_sb[:], axis=mybir.AxisListType.X,
                    op=mybir.AluOpType.add,
                )
                nc.vector.reciprocal(out=ss2[:], in_=ss2[:])
                find2_sb = singles.tile([K, S, N], HALF, name="find2")
                nc.vector.tensor_tensor(
                    out=find2_sb[:], in0=e2_sb[:],
                    in1=ss2[:].to_broadcast([K, S, N]),
                    op=mybir.AluOpType.mult,
                )
                for s in range(S):
                    st[s]["find"] = find2_sb[:, s, :]

            def stage_softmax(s):
                d = st[s]
                logits_ps = psum.tile([K, N], F32, tag="ps", name=f"lg_ps{s}")
                for c in range(4):
                    nc.tensor.matmul(
                        out=logits_ps[:],
                        lhsT=qT2_sb[:, c, d["ks"]],
                        rhs=ftT2_sb[:, c, d["ns"]],
                        start=(c == 0),
                        stop=(c == 3),
                    )
                if bm_ones:
                    lg_sb = logits_ps
                else:
                    lg_sb = ps.tile([K, N], F32, name=f"lg{s}", tag=f"lg{s}")
                    nc.vector.tensor_tensor(
                        out=lg_sb[:], in0=logits_ps[:], in1=d["bmmul"],
                        op=mybir.AluOpType.mult,
                    )
                    nc.vector.tensor_tensor(
                        out=lg_sb[:], in0=lg_sb[:], in1=d["bmadd"],
                        op=mybir.AluOpType.add,
                    )
                nmx = ps.tile([K, 1], F32, name=f"nmx{s}", tag=f"nmx{s}")
                nc.vector.tensor_reduce(
                    out=nmx[:], in_=lg_sb[:], axis=mybir.AxisListType.X,
                    op=mybir.AluOpType.max, negate=True,
                )
                e_sb = ps.tile([K, N], F32, name=f"e{s}", tag=f"e{s}")
                ssum = ps.tile([K, 1], F32, name=f"ss{s}", tag=f"ss{s}")
                nc.scalar.activation(
                    out=e_sb[:], in_=lg_sb[:],
                    func=mybir.ActivationFunctionType.Exp,
                    bias=nmx[:], scale=1.0, accum_out=ssum[:],
                )
                rs = ps.tile([K, 1], F32, name=f"rs{s}", tag=f"rs{s}")
                nc.vector.reciprocal(out=rs[:], in_=ssum[:])
                if not nm_ones:
                    nc.vector.tensor_tensor(
                        out=rs[:], in0=rs[:], in1=d["nmcol"], op=mybir.AluOpType.mult
                    )
                find_sb = ps.tile([K, N], HALF, name=f"find{s}", tag=f"find{s}")
                nc.vector.tensor_scalar_mul(out=find_sb[:], in0=e_sb[:], scalar1=rs[:])
                d["find"] = find_sb

            def stage_proj(s):
                d = st[s]
                find_sb = d["find"]
                gfT_ps = psum.tile([N, R], F32, tag="ps", name=f"gfT_ps{s}")
                nc.tensor.matmul(
                    out=gfT_ps[:], lhsT=find_sb[:], rhs=d["GTm"], start=True, stop=True
                )
                gfT_sb = ps.tile([N, R], HALF, name=f"gfT{s}", tag=f"gfT{s}")
                nc.vector.tensor_copy(out=gfT_sb[:], in_=gfT_ps[:])
                d["gfT"] = gfT_sb
                f2T_ps = psum.tile([N, K], F32, tag="ps", name=f"f2T_ps{s}")
                nc.tensor.matmul(
                    out=f2T_ps[:], lhsT=find_sb[:], rhs=I128_sb[:K, :K],
                    start=True, stop=False,
                )
                d["f2T_ps"] = f2T_ps

            def stage_fast_f2T(s):
                # find2 scatter via the host-folded GS[kk, (r, k)] =
                # 0.5*GT[kk, r]*ST[r, k]: one matmul lands gfS, then 12
                # ea-matmuls accumulate straight into f2T — no hT transpose
                # round-trip. The sigmoid's +0.5 affine half is the
                # host-computed row t[k] broadcast via a ones outer-product.
                d = st[s]
                gfS_ps = psum.tile([N, R * K], F32, tag="ps", name=f"gfS_ps{s}")
                nc.tensor.matmul(
                    out=gfS_ps[:], lhsT=d["find"], rhs=d["GSm"],
                    start=True, stop=True,
                )
                gfS_sb = ps.tile([N, R * K], HALF, name=f"gfS{s}", tag=f"gfS{s}")
                nc.vector.tensor_copy(out=gfS_sb[:], in_=gfS_ps[:])
                f2T_ps = psum.tile([N, K], F32, tag="ps", name=f"f2T_ps{s}")
                nc.tensor.matmul(
                    out=f2T_ps[:], lhsT=d["find"], rhs=I128_sb[:K, :K],
                    start=True, stop=False,
                )
                for r in range(R):
                    nc.tensor.matmul(
                        out=f2T_ps[:], lhsT=d["ea"][:, r, :],
                        rhs=gfS_sb[:, K * r : K * (r + 1)],
                        start=False, stop=False,
                    )
                nc.tensor.matmul(
                    out=f2T_ps[:], lhsT=ones128rh_sb[:, :N], rhs=d["tm"],
                    start=False, stop=True,
                )
                d["f2T_ps"] = f2T_ps

            def stage_edge(s):
                d = st[s]
                ea_all = ps.tile([N, R, N], HALF, name=f"ea{s}", tag=f"ea{s}")
                GR = R // 2
                for g in range(2):
                    Bg = psum.tile([N, GR, N], F32, tag="ps", name=f"B6_{s}{g}")
                    for i in range(GR):
                        r = GR * g + i
                        sel = I12h_sb[:, r : r + 1].to_broadcast([K, N])
                        nc.tensor.matmul(
                            out=Bg[:, i, :], lhsT=sel, rhs=d["A0"][:],
                            start=(i == 0), stop=False,
                        )
                        nc.tensor.matmul(
                            out=Bg[:, i, :], lhsT=d["A1"][:], rhs=sel,
                            start=False, stop=(i == GR - 1),
                        )
                    # sigmoid(x) = 0.5*tanh(x/2) + 0.5; tanh shares the Exp
                    # table set, so no act-table switch mid-chain. In the
                    # fast path the affine is folded into GS/t host-side.
                    nc.scalar.activation(
                        out=ea_all[:, GR * g : GR * (g + 1), :], in_=Bg[:],
                        func=mybir.ActivationFunctionType.Tanh,
                        scale=0.5,
                    )
                if not fast:
                    nc.vector.tensor_scalar(
                        out=ea_all[:], in0=ea_all[:],
                        scalar1=0.5, scalar2=0.5,
                        op0=mybir.AluOpType.mult, op1=mybir.AluOpType.add,
                    )
                    if not rm_ones:
                        nc.vector.tensor_tensor(
                            out=ea_all[:],
                            in0=ea_all[:],
                            in1=d["rmask"][:, None, :].to_broadcast([N, R, N]),
                            op=mybir.AluOpType.mult,
                        )
                d["ea"] = ea_all

            def stage_h(s):
                d = st[s]
                h_ps = psum.tile([R, N], F32, tag="ps", name=f"h_ps{s}")
                hT_ps = psum.tile([N, R], F32, tag="ps", name=f"hT_ps{s}")
                for r in range(R):
                    nc.tensor.matmul(
                        out=hT_ps[:, r : r + 1],
                        lhsT=d["ea"][:, r, :],
                        rhs=d["gfT"][:, r : r + 1],
                        start=(r == 0),
                        stop=(r == R - 1),
                    )
                hT_sb = ps.tile([N, R], HALF, name=f"hT{s}", tag=f"hT{s}")
                nc.scalar.copy(out=hT_sb[:], in_=hT_ps[:])
                nc.tensor.matmul(
                    out=h_ps[:], lhsT=hT_sb[:], rhs=I128_sb[:N, :N],
                    start=True, stop=True,
                )
                h_sb = ps.tile([R, N], HALF, name=f"h{s}", tag=f"h{s}")
                nc.vector.tensor_copy(out=h_sb[:], in_=h_ps[:])
                nc.tensor.matmul(
                    out=d["f2T_ps"][:], lhsT=h_sb[:], rhs=d["STm"],
                    start=False, stop=True,
                )

            def stage_fa_red(s):
                # fast path, DVE part: one reduce lands the raw fa column;
                # the peak-norm runs in parallel off-path and is applied to
                # outT in the epilogue (linear, so exact). s0's fa2 column is
                # at base partition 0, so the faT matmul can read it straight
                # back; s1's block starts at partition 64 (matmul operands
                # must share a base partition), so reduce to a scratch tile
                # and let Act copy it into fa2.
                d = st[s]
                if s == 0:
                    nc.vector.tensor_reduce(
                        out=fa2_sb[:N, 0:1],
                        in_=d["f2T_ps"][:], axis=mybir.AxisListType.X,
                        op=mybir.AluOpType.max,
                    )
                    d["fa_sb"] = fa2_sb[:N, 0:1]
                else:
                    fa_sb = ps.tile([N, 1], HALF, name=f"fa{s}", tag=f"fa{s}")
                    nc.vector.tensor_reduce(
                        out=fa_sb[:], in_=d["f2T_ps"][:], axis=mybir.AxisListType.X,
                        op=mybir.AluOpType.max,
                    )
                    nc.scalar.copy(
                        out=fa2_sb[N * s : N * (s + 1), s : s + 1], in_=fa_sb[:]
                    )
                    d["fa_sb"] = fa_sb

            def stage_fa_nr(s):
                # peak over the 64 boxes = a partition-axis (C) reduce, which
                # GPSIMD can do straight off the SBUF fa column — no PE
                # transpose, no DVE involvement
                d = st[s]
                nrv = ps.tile([N, 1], F32, name=f"nrv{s}", tag=f"nrv{s}")
                nc.gpsimd.partition_all_reduce(
                    nrv[:], d["fa_sb"][:], N, bass_isa.ReduceOp.max
                )
                nr = nrv[0:1, :]
                nc.gpsimd.tensor_scalar_max(out=nr, in0=nr, scalar1=1.0)
                nc.vector.reciprocal(out=nr2row_sb[:, s : s + 1], in_=nr)

            def stage_fa_slow(s):
                d = st[s]
                # slow path (untimed): reference order is normalize, then
                # mask; do it on the row and transpose back; epilogue scale
                # is neutered (nr2 <- 1)
                fa_sb = ps.tile([N, 1], HALF, name=f"fa{s}", tag=f"fa{s}")
                nc.vector.tensor_reduce(
                    out=fa_sb[:], in_=d["f2T_ps"][:], axis=mybir.AxisListType.X,
                    op=mybir.AluOpType.max,
                )
                faT_ps = psum.tile([1, N], F32, tag="ps", name=f"faT_ps{s}")
                nc.tensor.matmul(
                    out=faT_ps[:], lhsT=fa_sb[:], rhs=I128_sb[:N, :N],
                    start=True, stop=True,
                )
                nr = ps.tile([1, 1], F32, name=f"nr{s}", tag=f"nr{s}")
                nc.vector.tensor_reduce(
                    out=nr[:], in_=faT_ps[:], axis=mybir.AxisListType.X,
                    op=mybir.AluOpType.max,
                )
                nc.vector.tensor_scalar_max(out=nr[:], in0=nr[:], scalar1=1.0)
                nc.vector.reciprocal(out=nr[:], in_=nr[:])
                nc.vector.memset(nr2row_sb[:, s : s + 1], 1.0)
                faT_sb = ps.tile([1, N], HALF, name=f"faT{s}", tag=f"faT{s}")
                nc.vector.tensor_scalar_mul(
                    out=faT_sb[:], in0=faT_ps[:], scalar1=nr[:]
                )
                nc.vector.tensor_tensor(
                    out=faT_sb[:], in0=faT_sb[:], in1=d["famul"],
                    op=mybir.AluOpType.mult,
                )
                nc.vector.tensor_tensor(
                    out=faT_sb[:], in0=faT_sb[:], in1=d["faadd"],
                    op=mybir.AluOpType.add,
                )
                fac_ps = psum.tile([N, 1], F32, tag="ps", name=f"fac_ps{s}")
                nc.tensor.matmul(
                    out=fac_ps[:], lhsT=faT_sb[:], rhs=ones1_sb[:, :1],
                    start=True, stop=True,
                )
                nc.scalar.copy(
                    out=fa2_sb[N * s : N * (s + 1), s : s + 1], in_=fac_ps[:]
                )

            # NOTE: do NOT prefetch the Sigmoid table early — the Act engine
            # holds two resident sets (sqrt+exp by now) and tanh shares the
            # exp set, so no mid-chain reloads occur.
            if fast:
                softmax2()
                for s in range(S):
                    stage_edge(s)
                for s in range(S):
                    stage_fast_f2T(s)
                    stage_fa_red(s)
                    stage_fa_nr(s)
            else:
                for s in range(S):
                    stage_softmax(s)
                for s in range(S):
                    stage_proj(s)
                for s in range(S):
                    stage_edge(s)
                for s in range(S):
                    stage_h(s)
                if bm_ones:
                    for s in range(S):
                        stage_fa_red(s)
                    for s in range(S):
                        stage_fa_nr(s)
                else:
                    for s in range(S):
                        stage_fa_slow(s)

            # w_c = visf_c^T @ fa2 = the mem chunk; split per sample so
            # sample 0's w and outT half run before sample 1's fa lands
            w_ps = psum.tile([P, len(W_CHUNKS), S], F32, tag="ps", name="w_ps")
            w_all = singles.tile([P, len(W_CHUNKS), S], HALF)

            # broadcast the peak-norm reciprocals across partitions
            nrbc_ps = psum.tile([P, S], F32, tag="ps", name="nrbc_ps")
            nc.tensor.matmul(
                out=nrbc_ps[:], lhsT=ones128r_sb[:], rhs=nr2row_sb[:],
                start=True, stop=True,
            )
            # the copy folds the fp8 W_out de-quantization (1/s_wo)
            nrbc_sb = singles.tile([P, S], F32)
            nc.vector.tensor_scalar_mul(
                out=nrbc_sb[:], in0=nrbc_ps[:], scalar1=1.0 / s_wo
            )

            # ---- tail in transposed [128, (j, s)] layout: outT[p, j, s] =
            # (sum_c sum_v WoT[v, c, 128j+p]*w[v, c, s]) * nr[s] + b[128j+p].
            # c-outer so only chunk 14/15's 8 tiny matmuls trail the last
            # WoT byte. All epilogue ops run on 128 partitions. ----
            outT_ps = psumO.tile([P, 8, S], F32, tag="outT", name="outT_ps")
            for s in range(S):
                for i in range(len(W_CHUNKS)):
                    nc.tensor.matmul(
                        out=w_ps[:, i, s : s + 1], lhsT=visfT_sb[i][:],
                        rhs=fa2_sb[:, s : s + 1], start=True, stop=True,
                    )
                if s == 0:
                    nc.vector.tensor_copy(
                        out=w_all[:, :, 0:1], in_=w_ps[:, :, 0:1]
                    )
                else:
                    nc.scalar.copy(out=w_all[:, :, 1:2], in_=w_ps[:, :, 1:2])
                for j in range(8):
                    for i, c in enumerate(W_CHUNKS):
                        nc.tensor.matmul(
                            out=outT_ps[:, j, s : s + 1],
                            lhsT=WoT_sb[:, c, P * j : P * (j + 1)],
                            rhs=w_all[:, i, s : s + 1],
                            start=(i == 0), stop=(i == len(W_CHUNKS) - 1),
                        )
            outT_sb = singles.tile([P, 8, S], F32)
            nc.vector.tensor_tensor(
                out=outT_sb[:], in0=outT_ps[:],
                in1=nrbc_sb[:, None, :].to_broadcast([P, 8, S]),
                op=mybir.AluOpType.mult,
            )
            nc.vector.tensor_tensor(
                out=outT_sb[:], in0=outT_sb[:], in1=boutT_sb[:],
                op=mybir.AluOpType.add,
            )
            nc.sync.dma_start(
                out=d_out[:], in_=outT_sb[:].rearrange("p j s -> p (j s)")
            )

    nc.finalize()
    return nc


def _host_prep(inputs):
    node_rep = np.asarray(inputs["node_rep"], np.float32)
    relate_rep = np.asarray(inputs["relate_rep"], np.float32)
    relate_os = np.asarray(inputs["relate_os"])
    relate_mask = np.asarray(inputs["relate_mask"], np.float32)
    vision_feat = np.asarray(inputs["vision_feat"], np.float32)
    relation_mask = np.asarray(inputs["relation_mask"], np.float32)
    box_mask = np.asarray(inputs["box_mask"], np.float32)
    node_mask = np.asarray(inputs["node_mask"], np.float32)
    norm_w = np.asarray(inputs["norm_w"], np.float32)
    W_v = np.asarray(inputs["W_v"], np.float32)
    W_e = np.asarray(inputs["W_e"], np.float32)
    W_node = np.asarray(inputs["W_node"], np.float32)
    W_rel = np.asarray(inputs["W_rel"], np.float32)
    W_out = np.asarray(inputs["W_out"], np.float32)
    b_out = np.asarray(inputs["b_out"], np.float32)

    s_mean = np.float32(np.mean(norm_w))
    WvT = (W_v.T * s_mean).astype(np.float32)
    WnT = (W_node.T / np.float32(np.sqrt(DV))).astype(np.float32)
    WA0 = (W_rel.T @ W_e[:, :DV] / np.float32(np.sqrt(DE))).astype(np.float32)
    WA1 = (W_rel.T @ W_e[:, DV:] / np.float32(np.sqrt(DE))).astype(np.float32)
    WoT = np.ascontiguousarray(W_out.T)
    s_wv = _pow2_scale(WvT)
    s_wn = _pow2_scale(WnT)
    s_a0 = _pow2_scale(WA0)
    s_a1 = _pow2_scale(WA1)
    s_wo = _pow2_scale(WoT)
    scales = (s_wv, s_wn, s_a0, s_a1, s_wo)

    subj = relate_os[..., 1].astype(np.int64)
    obj = relate_os[..., 0].astype(np.int64)
    valid = (subj != -1).astype(np.float32)
    obj_c = np.clip(obj, 0, K - 1)
    subj_c = np.clip(subj, 0, K - 1)
    G = np.zeros((B, R, K), np.float32)
    STm = np.zeros((B, R, K), np.float32)
    bi = np.arange(B)[:, None]
    ri = np.arange(R)[None, :]
    G[bi, ri, obj_c] = valid * relate_mask
    STm[bi, ri, subj_c] = 1.0

    bmmul = (box_mask > 0).astype(np.float32)
    bmadd = (bmmul - 1.0) * np.float32(6e4)  # fp16-safe large negative
    famul = box_mask
    faadd = (1.0 - box_mask) * np.float32(1e-7)

    WvT_p = _pack(WvT * s_wv).astype(E3NP)
    WoT_p = _pack(WoT * s_wo).astype(E3NP)
    wq_p = np.concatenate(
        [_pack(WnT * s_wn), _pack(WA0 * s_a0), _pack(WA1 * s_a1)], axis=1
    ).astype(E3NP)
    I128 = np.eye(P, dtype=np.float32)
    # boutT[p, (j, s)] = b_out[128j+p], duplicated across the s positions
    bout2 = np.ascontiguousarray(
        np.repeat(b_out.reshape(8, P).T[:, :, None], S, axis=2).reshape(P, 16)
    ).astype(np.float16)

    fast = (
        bool(np.all(box_mask == 1.0))
        and bool(np.all(node_mask == 1.0))
        and bool(np.all(relation_mask == 1.0))
    )
    smf = (R * K + K) if fast else SMALLS_F

    def smalls_for(b):
        sm = np.zeros((N, smf), np.float32)
        if fast:
            # GS[kk, r*K + k] = 0.5*GT[kk, r]*ST[r, k] fuses the obj-gather,
            # the subj-scatter, and the 0.5 tanh scale into one operand; the
            # +0.5 sigmoid affine reduces to the host row t[k] (rowsum(find)
            # = node_mask = 1 in the fast path)
            GS = 0.5 * G[b].T[:, :, None] * STm[b][None, :, :]  # [K, R, K]
            sm[:K, 0 : R * K] = GS.reshape(K, R * K)
            sm[0, R * K : R * K + K] = 0.5 * G[b].sum(axis=1) @ STm[b]
            return sm
        sm[:, _SM_RM : _SM_RM + N] = relation_mask[b]
        sm[:K, _SM_BM : _SM_BM + N] = bmmul[b][None, :]
        sm[:K, _SM_BA : _SM_BA + N] = bmadd[b][None, :]
        sm[:K, _SM_GT : _SM_GT + R] = G[b].T
        sm[:R, _SM_ST : _SM_ST + K] = STm[b]
        sm[:K, _SM_NM] = node_mask[b]
        sm[0, _SM_FM : _SM_FM + N] = famul[b]
        sm[0, _SM_FA : _SM_FA + N] = faadd[b]
        return sm

    in_maps = []
    for c in range(NCORES):
        b0 = S * c
        visf2 = np.concatenate(
            [_pack(vision_feat[b]).reshape(P, 16, N) for b in range(b0, b0 + S)],
            axis=2,
        ).reshape(P, -1)
        nrep2 = np.concatenate(
            [
                _pack(np.ascontiguousarray(node_rep[b].T)).reshape(P, 4, K)
                for b in range(b0, b0 + S)
            ],
            axis=2,
        ).reshape(P, -1)
        rrep2 = np.concatenate(
            [
                _pack(np.ascontiguousarray(relate_rep[b].T)).reshape(P, 4, R)
                for b in range(b0, b0 + S)
            ],
            axis=2,
        ).reshape(P, -1)
        I12blk = np.zeros((P, K), np.float32)
        I12blk[:K, :K] = np.eye(K, dtype=np.float32)
        wcat_full = np.ascontiguousarray(
            np.concatenate([nrep2, rrep2, I12blk], axis=1)
        )
        smalls2 = np.concatenate(
            [smalls_for(b) for b in range(b0, b0 + S)], axis=1
        )  # [64, S*smf]
        rest32 = np.zeros((P, P + S * smf), np.float32)
        rest32[:, :P] = I128
        rest32[:N, P:] = smalls2
        m = {
            "visf16": np.ascontiguousarray(visf2).astype(np.float16),
            "WvT": WvT_p,
            "wcat": wcat_full.astype(np.float16),
            "wq": wq_p,
            "WoT": WoT_p,
            "bout": bout2,
            "resth": rest32.astype(np.float16),
        }
        in_maps.append(m)
    return in_maps, scales


def kernel(**inputs) -> np.ndarray:
    bm_ones = bool(np.all(np.asarray(inputs["box_mask"]) == 1.0))
    nm_ones = bool(np.all(np.asarray(inputs["node_mask"]) == 1.0))
    rm_ones = bool(np.all(np.asarray(inputs["relation_mask"]) == 1.0))
    in_maps, scales = _host_prep(inputs)
    key = ("nc", bm_ones, nm_ones, rm_ones, scales)
    if key not in _cache:
        _cache[key] = build_nc(bm_ones, nm_ones, rm_ones, scales)
    nc = _cache[key]
    res = run_bass_kernel_spmd(nc, in_maps, core_ids=list(range(NCORES)))
    outs = []
    for c in range(NCORES):
        t = np.asarray(res.results[c]["out"]).reshape(P, 8, S)
        outs.append(t.transpose(2, 1, 0).reshape(S, DC))  # out[s,128j+p]
    return np.concatenate(outs, axis=0).astype(np.float32)



# revision 42
# speedup vs baseline: 1.1261x; 1.1261x over previous
"""Trainium2 Bass kernel for nn_CMR_59931973648949 (gnn_message_passing).

Contract: kernel(**inputs) takes FULL unsharded numpy inputs and returns the
FULL [16, 1024] output. Data-parallel over batch across 8 cores (2 samples
per core, weights replicated). All weights are host-packed partition-major
([128, F], fp16) for max-bandwidth DMAs; the two local samples are batched
through the shared heavy matmuls (feat_v, q/u0/u1 projections). The ~9.8MB
weight stream (dominated by W_out 4MB + W_v 2MB fp16) is the kernel floor;
everything else is arranged to hide under it:

- Readout restructure: YT[n, dc] = sum_v visf[v, n]*WoT[v, dc] accumulates
  chunk-by-chunk as the WoT stream lands (independent of the chain result);
  the tail is outT[p, j, s] = YT^T-slices @ fa2 in a [128, (j, s)] layout so
  every epilogue op runs on 128 partitions. The last two WoT v-chunks
  (12..15) skip YT and enter outT directly via w_c = visf_c^T @ fa2, so no
  PSUM->SBUF copy of YT trails the last DMA byte.
- Act table discipline: the engine holds one resident set and a reload is
  ~1.3us, so the edge sigmoid is computed as 0.5*tanh(x/2)+0.5 (tanh shares
  the Exp set). The affine half folds into gfT (x0.5) and a k=1 crow matmul;
  a dummy Exp anchored after the sqrt pins the load order sqrt->exp with no
  mid-chain switches.
- The final attention peak-norm is a GPSIMD partition-axis (C) reduce on the
  SBUF fa column (no PE transpose), and the 1/nr scale is applied in the
  epilogue via a PE ones-broadcast (nrbc), keeping the chain short.

Math per sample (see reference):
  scl[n] = mean(norm_w)/max(||visf[:,n]||,1e-12)   (folded into feat_v scale)
  feat_v = (visf.T * scl) @ W_v.T ; used only via feat_vT
  q/u0/u1 from node/relate reps with WnT=W_node.T/sqrt(DV),
      WA0/1=W_rel.T@W_e[:, :DV | DV:]/sqrt(DE)
  find = softmax(mask(q @ feat_vT)) * node_mask
  ea_r = sigmoid(A0[r,:] bcast + A1T[:,r]) * relation_mask
  g_findT = find.T-gather via GT (folds valid*relate_mask*onehot(obj))
  h[r,:] = g_find[r,:] @ ea_r ; find2T = findT + h.T @ ST (onehot(subj))
  fa = rowmax(find2T); fa /= max(max(fa),1); fa = fa*bm + (1-bm)*1e-7
  out[s, 128j+p] = outT[p, j, s] = (fa2^T YT + w^T WoT)*nr + b_out
"""

import numpy as np

import concourse.bass as bass
from concourse import bass_isa
import concourse.tile as tile
from concourse import bacc, mybir
from concourse.bass_utils import run_bass_kernel_spmd

P = 128
B, K, R, N = 16, 12, 12, 64
DW, DV, DVIS, DE, DC = 512, 512, 2048, 512, 1024
NCORES = 8
S = B // NCORES  # samples per core = 2
N2 = S * N  # 128: both samples' boxes side by side
K2 = S * K  # 24

F32 = mybir.dt.float32
F32R = mybir.dt.float32r
BF16 = mybir.dt.bfloat16
F16 = mybir.dt.float16
E3 = mybir.dt.float8e3  # fp8 e3m4: 4-bit mantissa, max 15.5
USE_F32R = True
HALF = F16        # half dtype for weight DMAs (fp16: 10-bit mantissa)
WVT_BF16 = True   # feat_v matmul operands in half
WCAT_BF16 = True  # q/u0/u1 weight + reps in bf16
WOT_BF16 = True   # W_out matmul operands in half (fp16 keeps ~3e-4)
import ml_dtypes

E3NP = ml_dtypes.float8_e3m4
E3_TARGET = 8.0   # quantized absmax target (<= 15.5 with 2x headroom)


def _pow2_scale(a):
    """Power-of-2 scale s s.t. absmax(a*s) lands just under E3_TARGET."""
    amax = float(np.abs(a).max())
    return float(2.0 ** np.floor(np.log2(E3_TARGET / max(amax, 1e-30))))

# smalls packing column offsets (per sample, [64, SMALLS_F])
_SM_RM = 0          # rmask      [64, 64]
_SM_BM = 64         # bmmul      [12, 64]
_SM_BA = 128        # bmadd      [12, 64]
_SM_GT = 192        # GT         [12, 12]
_SM_ST = 204        # ST         [12, 12]
_SM_NM = 216        # nmcol      [12, 1]
_SM_FM = 217        # famul row  [1, 64]
_SM_FA = 281        # faadd row  [1, 64]
SMALLS_F = 345

_cache = {}


def _pack(a):
    """[(o*128), F] row-major -> [128, o*F] partition-major."""
    o = a.shape[0] // P
    return np.ascontiguousarray(
        a.reshape(o, P, a.shape[1]).transpose(1, 0, 2).reshape(P, -1)
    )


def build_nc(bm_ones=False, nm_ones=False, rm_ones=False,
             scales=(1.0, 1.0, 1.0, 1.0, 1.0)):
    # scales = (s_wv, s_wn, s_a0, s_a1): power-of-2 fp8 quantization scales
    # baked into the weight streams; de-scaled at the existing PSUM->SBUF
    # copies (q/u0/u1 pre-compensate feat_v's s_wv since logits/A0/A1 are
    # the only consumers of feat_v).
    s_wv, s_wn, s_a0, s_a1, s_wo = scales
    nc = bacc.Bacc(num_devices=NCORES)

    FR = F32R if USE_F32R else F32
    d_visf = nc.declare_dram_parameter("visf16", [P, 16 * N2], HALF, isOutput=False)
    d_WvT = nc.declare_dram_parameter("WvT", [P, 16 * DV], E3, isOutput=False)
    # wcat = nrepT2 | rrepT2 | I12 (fp16); wq = WnT | WA0 | WA1 (fp8 e3m4)
    WCATF = 2 * 4 * K2 + K
    d_wcat = nc.declare_dram_parameter("wcat", [P, WCATF], HALF, isOutput=False)
    d_wq = nc.declare_dram_parameter("wq", [P, 3 * 4 * DV], E3, isOutput=False)
    d_WoT = nc.declare_dram_parameter("WoT", [P, 16 * DC], E3, isOutput=False)
    # bias transposed to the outT layout: boutT[p, (j, s)] = b_out[128j+p]
    d_bout = nc.declare_dram_parameter("bout", [P, 16], HALF, isOutput=False)
    # resth = I128 [P, 128] | smalls [64, S*smf]  (all fp16). In the
    # all-ones fast path only GT/ST are consumed, so the smalls shrink from
    # 345 to 24 columns per sample (~82KB less stream traffic).
    fast = bm_ones and nm_ones and rm_ones
    # fast-path smalls: GS [K, R*K] at 0, t row [1, K] at R*K
    smf = (R * K + K) if fast else SMALLS_F
    d_rest = nc.declare_dram_parameter(
        "resth", [P, P + S * smf], HALF, isOutput=False
    )
    # transposed output: d_out[p, 2j+s] = out[s, 128j+p]; host un-permutes
    d_out = nc.declare_dram_parameter("out", [P, 16], F32, isOutput=True)

    with tile.TileContext(nc) as tc:
        with (
            tc.tile_pool(name="singles", bufs=1) as singles,
            tc.tile_pool(name="ps", bufs=2) as ps,
            tc.tile_pool(name="psum", bufs=6, space="PSUM") as psum,
            tc.tile_pool(name="psumO", bufs=1, space="PSUM") as psumO,
        ):
            # ---- DMAs on the critical path first (SP queue runs in order) ----
            visf2_mm = singles.tile([P, 16, N2], HALF)
            nc.sync.dma_start(
                out=visf2_mm[:], in_=d_visf[:].rearrange("p (o n) -> p o n", o=16)
            )
            # I128 gates the scl chain and every PE transpose — land it first
            rest_sb = singles.tile([P, P + S * smf], HALF)
            nc.sync.dma_start(out=rest_sb[:, :P], in_=d_rest[:, :P])
            # reps+WnT land before the big W_v stream: the q/u0/u1 gate is
            # the chain-start bottleneck, feat_v is not
            wcatall_sb = singles.tile([P, WCATF], HALF)
            nc.sync.dma_start(out=wcatall_sb[:], in_=d_wcat[:])
            WvT_sb = singles.tile([P, 16, DV], E3)
            for g in range(2):
                nc.sync.dma_start(
                    out=WvT_sb[:, 8 * g : 8 * g + 8, :],
                    in_=d_WvT[:, 8 * g * DV : 8 * (g + 1) * DV].rearrange(
                        "p (o d) -> p o d", o=8
                    ),
                )
            wq_sb = singles.tile([P, 3, 4, DV], E3)
            nc.sync.dma_start(
                out=wq_sb[:, 0],
                in_=d_wq[:, : 4 * DV].rearrange("p (o d) -> p o d", o=4),
            )
            nc.sync.dma_start(
                out=wq_sb[:, 1:],
                in_=d_wq[:, 4 * DV :].rearrange("p (t o d) -> p t o d", t=2, o=4),
            )
            # W_out weight stream: 7 transfers of 2 v-chunks, then c14 and
            # c15 alone — the final two chunks feed the outT accumulation
            # directly (w-trick) so no YT copy trails the stream. The smalls
            # (masks/GT/ST) and bias slot in after the first WoT group: they
            # are not needed until the proj stage (~17us), and keeping them
            # out of the front moves the last WoT byte ~0.5us earlier.
            WoT_sb = singles.tile([P, 16, DC], E3)
            boutT_sb = singles.tile([P, 8, S], HALF)
            for g in range(7):
                nc.sync.dma_start(
                    out=WoT_sb[:, 2 * g : 2 * g + 2, :],
                    in_=d_WoT[:, 2 * g * DC : 2 * (g + 1) * DC].rearrange(
                        "p (o d) -> p o d", o=2
                    ),
                )
                if g == 0:
                    nc.sync.dma_start(out=rest_sb[:, P:], in_=d_rest[:, P:])
                    nc.sync.dma_start(
                        out=boutT_sb[:],
                        in_=d_bout[:].rearrange("p (j s) -> p j s", j=8),
                    )
            for c in (14, 15):
                nc.sync.dma_start(
                    out=WoT_sb[:, c : c + 1, :],
                    in_=d_WoT[:, c * DC : (c + 1) * DC].rearrange(
                        "p (o d) -> p o d", o=1
                    ),
                )

            I128_sb = rest_sb[:, :P]
            smalls_sb = [
                rest_sb[:N, P + s * smf : P + (s + 1) * smf]
                for s in range(S)
            ]
            reps_sb = wcatall_sb[:, : 2 * 4 * K2].rearrange(
                "p (t o k) -> p t o k", t=2, o=4
            )
            I12h_sb = wcatall_sb[:K, 2 * 4 * K2 : 2 * 4 * K2 + K]
            WnT_sb = wq_sb[:, 0]
            WA0_sb = wq_sb[:, 1]
            WA1_sb = wq_sb[:, 2]

            nrep2 = reps_sb[:, 0]  # [P, 4, 24]
            rrep2 = reps_sb[:, 1]
            HALF_SM = HALF if WCAT_BF16 else F32  # dtype of small attention mms

            # fa2[:, s] holds sample s's (normalized, masked) final
            # attention column; zero outside its 64-row block
            fa2_sb = singles.tile([N2, S], HALF)
            nc.gpsimd.memset(fa2_sb[:], 0.0)
            ones1_sb = singles.tile([1, S], HALF)
            nc.gpsimd.memset(ones1_sb[:], 1.0)
            # peak-norm reciprocals as a [1, S] row; broadcast across the
            # 128 partitions via a PE ones-outer-product for the epilogue
            nr2row_sb = singles.tile([1, S], F32)
            ones128r_sb = singles.tile([1, P], F32)
            nc.gpsimd.memset(ones128r_sb[:], 1.0)
            ones128c_sb = singles.tile([P, 1], HALF)
            nc.gpsimd.memset(ones128c_sb[:], 1.0)
            ones128rh_sb = singles.tile([1, P], HALF)
            nc.gpsimd.memset(ones128rh_sb[:], 1.0)

            # The readout is a pure w-trick: w_c = visfT_c @ fa2 is the
            # mem chunk, and outT accumulates WoT_c^T @ w_c as soon as fa2
            # is ready — no YT intermediate, nothing big trails the stream.
            # The 16 PE transposes also double as the PE p-state warmup (the
            # cost model needs ~3us of continuous PE activity before full
            # rate); copies go DVE-only so the Act table stays on sqrt->exp,
            # and a dedicated PSUM pool keeps featv_ps allocation unblocked.
            W_CHUNKS = tuple(range(16))
            visfT_sb = []
            for c in W_CHUNKS:
                vT_ps = psum.tile([N2, P], F32, tag="ps", name=f"vT_ps{c}")
                nc.tensor.matmul(
                    out=vT_ps[:], lhsT=visf2_mm[:, c, :], rhs=I128_sb[:],
                    start=True, stop=True,
                )
                vT = singles.tile([N2, P], HALF, name=f"vT{c}")
                if c % 2 == 0:
                    nc.scalar.copy(out=vT[:], in_=vT_ps[:])
                else:
                    nc.vector.tensor_copy(out=vT[:], in_=vT_ps[:])
                visfT_sb.append(vT)

            # ---- column norms: elementwise square (DVE) then per-chunk
            # ones-column matmuls accumulate the partition sums — only the
            # diagonal of the old gram product, at ~1/4 the PE time ----
            sq_sb = singles.tile([P, 16, N2], HALF)
            nc.vector.tensor_tensor(
                out=sq_sb[:], in0=visf2_mm[:], in1=visf2_mm[:],
                op=mybir.AluOpType.mult,
            )
            nrm2_ps = psum.tile([N2, 1], F32, tag="ps", name="nrm2")
            for c in range(16):
                nc.tensor.matmul(
                    out=nrm2_ps[:],
                    lhsT=sq_sb[:, c, :],
                    rhs=ones128c_sb[:],
                    start=(c == 0),
                    stop=(c == 15),
                )
            scl = singles.tile([N2, 1], F32)
            nc.scalar.sqrt(out=scl[:], in_=nrm2_ps[:])
            nc.vector.tensor_scalar_max(out=scl[:], in0=scl[:], scalar1=1e-12)
            nc.vector.reciprocal(out=scl[:], in_=scl[:])
            # prefetch the Exp/Tanh table right after the sqrt: the Act table
            # slot is single; sqrt->exp is the only transition and both
            # loads hide under the weight stream. The edge nonlinearity uses
            # tanh (same set) — sigmoid's set is never touched.
            dummy_sb = singles.tile([1, 1], F32)
            nc.scalar.activation(
                out=dummy_sb[:], in_=scl[:1, :],
                func=mybir.ActivationFunctionType.Exp,
            )

            # ---- shared: qT/u0T/u1T for both samples [d, 24] ----
            def lin_T(w_sb, x_ap, name, unscale, eng="act"):
                out_ps = psum.tile([P, 4, K2], F32, tag="ps", name=name + "_ps")
                for dc in range(4):
                    for wc in range(4):
                        nc.tensor.matmul(
                            out=out_ps[:, dc, :],
                            lhsT=w_sb[:, wc, P * dc : P * (dc + 1)],
                            rhs=x_ap[:, wc, :],
                            start=(dc == 0 and wc == 0),
                            stop=(dc == 3 and wc == 3),
                        )
                out_sb = singles.tile([P, 4, K2], HALF_SM, name=name)
                # the copy doubles as fp8 de-quantization (own scale and
                # feat_v's s_wv, since the product q.feat_v must be exact)
                if eng == "act":
                    nc.scalar.mul(out=out_sb[:], in_=out_ps[:], mul=unscale)
                else:
                    nc.vector.tensor_scalar_mul(
                        out=out_sb[:], in0=out_ps[:], scalar1=unscale
                    )
                return out_sb

            # ---- shared: feat_v for both samples [n2, 512], blocked by
            # 128-column output group so each group's scale-copy (Act/DVE
            # alternating), transpose, and ftT2 copy pipeline behind the
            # matmuls instead of waiting for the full 512 columns ----
            featv_ps = psum.tile([N2, DV], F32, tag="ps")
            for c in range(16):
                nc.tensor.matmul(
                    out=featv_ps[:],
                    lhsT=visf2_mm[:, c, :],
                    rhs=WvT_sb[:, c, :],
                    start=(c == 0),
                    stop=(c == 15),
                )
            # the copy applies the norm scale (per-partition scl); split
            # across Act and DVE so both halves land in parallel
            featv_sb = singles.tile([N2, DV], HALF)
            nc.scalar.mul(
                out=featv_sb[:, : DV // 2], in_=featv_ps[:, : DV // 2],
                mul=scl[:],
            )
            nc.vector.tensor_scalar_mul(
                out=featv_sb[:, DV // 2 :], in0=featv_ps[:, DV // 2 :],
                scalar1=scl[:],
            )
            qT2_sb = lin_T(WnT_sb, nrep2, "qT2", 1.0 / (s_wn * s_wv))
            # ftT2 transposes interleave with the u0/u1 projections on PE so
            # the A0/A1 gate (ftT2 copies + u copies) closes ~0.5us earlier
            ftT2_ps = psum.tile([P, 4, N2], F32, tag="ps")
            ftT2_sb = singles.tile([P, 4, N2], HALF_SM)

            def ftT2_T(c):
                nc.tensor.matmul(
                    out=ftT2_ps[:, c, :],
                    lhsT=featv_sb[:, P * c : P * (c + 1)],
                    rhs=I128_sb[:],
                    start=True,
                    stop=True,
                )

            ftT2_T(0)
            ftT2_T(1)
            nc.scalar.copy(out=ftT2_sb[:, :2], in_=ftT2_ps[:, :2])
            u0T2_sb = lin_T(WA0_sb, rrep2, "u0T2", 1.0 / (s_a0 * s_wv), "dve")
            ftT2_T(2)
            ftT2_T(3)
            nc.vector.tensor_copy(out=ftT2_sb[:, 2:], in_=ftT2_ps[:, 2:])
            u1T2_sb = lin_T(WA1_sb, rrep2, "u1T2", 1.0 / (s_a1 * s_wv))

            # ---- per-sample pipeline, stages interleaved across samples ----
            st = [dict() for _ in range(S)]
            for s in range(S):
                sm = smalls_sb[s]
                if fast:
                    st[s]["GSm"] = sm[:K, 0 : R * K]
                    st[s]["tm"] = sm[:1, R * K : R * K + K]
                else:
                    st[s]["GTm"] = sm[:K, _SM_GT : _SM_GT + R]
                    st[s]["STm"] = sm[:R, _SM_ST : _SM_ST + K]
                if not fast:
                    st[s]["rmask"] = sm[:, _SM_RM : _SM_RM + N]
                    st[s]["bmmul"] = sm[:K, _SM_BM : _SM_BM + N]
                    st[s]["bmadd"] = sm[:K, _SM_BA : _SM_BA + N]
                    st[s]["nmcol"] = sm[:K, _SM_NM : _SM_NM + 1]
                    st[s]["famul"] = sm[:1, _SM_FM : _SM_FM + N]
                    st[s]["faadd"] = sm[:1, _SM_FA : _SM_FA + N]
                st[s]["ks"] = slice(K * s, K * (s + 1))
                st[s]["ns"] = slice(N * s, N * (s + 1))

            # ---- shared: A0/A1 for both samples, batched [R, S, N] (they
            # depend only on u0/u1 + ftT2, not on find — so the edge Bg and
            # the Act-serial sigmoid chain can start before the softmax) ----
            A0_ps = psum.tile([R, S, N], F32, tag="ps", name="A0_ps")
            A1_ps = psum.tile([R, S, N], F32, tag="ps", name="A1_ps")
            for u_sb, ps_t in ((u0T2_sb, A0_ps), (u1T2_sb, A1_ps)):
                for s in range(S):
                    d = st[s]
                    for c in range(4):
                        nc.tensor.matmul(
                            out=ps_t[:, s, :], lhsT=u_sb[:, c, d["ks"]],
                            rhs=ftT2_sb[:, c, d["ns"]],
                            start=(c == 0), stop=(c == 3),
                        )
            A0_sb = singles.tile([R, S, N], HALF_SM, name="A0")
            nc.scalar.copy(out=A0_sb[:], in_=A0_ps[:])
            A1_sb = singles.tile([R, S, N], HALF_SM, name="A1")
            nc.vector.tensor_copy(out=A1_sb[:], in_=A1_ps[:])
            for s in range(S):
                st[s]["A0"] = A0_sb[:, s, :]
                st[s]["A1"] = A1_sb[:, s, :]

            def softmax2():
                # fast path: both samples in one pass. Logits are O(1), so
                # exp runs without the max-subtract (mathematically equal)
                lg2_ps = psum.tile([K, S, N], F32, tag="ps", name="lg2")
                for s in range(S):
                    d = st[s]
                    for c in range(4):
                        nc.tensor.matmul(
                            out=lg2_ps[:, s, :], lhsT=qT2_sb[:, c, d["ks"]],
                            rhs=ftT2_sb[:, c, d["ns"]],
                            start=(c == 0), stop=(c == 3),
                        )
                e2_sb = singles.tile([K, S, N], F32, name="e2")
                nc.scalar.activation(
                    out=e2_sb[:], in_=lg2_ps[:],
                    func=mybir.ActivationFunctionType.Exp,
                )
                ss2 = singles.tile([K, S, 1], F32, name="ss2")
                nc.vector.tensor_reduce(
                    out=ss2[:], in_=e2_sb[:], axis=mybir.AxisListType.X,
                    op=mybir.AluOpType.add,
                )
                nc.vector.reciprocal(out=ss2[:], in_=ss2[:])
                find2_sb = singles.tile([K, S, N], HALF, name="find2")
                nc.vector.tensor_tensor(
                    out=find2_sb[:], in0=e2_sb[:],
                    in1=ss2[:].to_broadcast([K, S, N]),
                    op=mybir.AluOpType.mult,
                )
                for s in range(S):
                    st[s]["find"] = find2_sb[:, s, :]

            def stage_softmax(s):
                d = st[s]
                logits_ps = psum.tile([K, N], F32, tag="ps", name=f"lg_ps{s}")
                for c in range(4):
                    nc.tensor.matmul(
                        out=logits_ps[:],
                        lhsT=qT2_sb[:, c, d["ks"]],
                        rhs=ftT2_sb[:, c, d["ns"]],
                        start=(c == 0),
                        stop=(c == 3),
                    )
                if bm_ones:
                    lg_sb = logits_ps
                else:
                    lg_sb = ps.tile([K, N], F32, name=f"lg{s}", tag=f"lg{s}")
                    nc.vector.tensor_tensor(
                        out=lg_sb[:], in0=logits_ps[:], in1=d["bmmul"],
                        op=mybir.AluOpType.mult,
                    )
                    nc.vector.tensor_tensor(
                        out=lg_sb[:], in0=lg_sb[:], in1=d["bmadd"],
                        op=mybir.AluOpType.add,
                    )
                nmx = ps.tile([K, 1], F32, name=f"nmx{s}", tag=f"nmx{s}")
                nc.vector.tensor_reduce(
                    out=nmx[:], in_=lg_sb[:], axis=mybir.AxisListType.X,
                    op=mybir.AluOpType.max, negate=True,
                )
                e_sb = ps.tile([K, N], F32, name=f"e{s}", tag=f"e{s}")
                ssum = ps.tile([K, 1], F32, name=f"ss{s}", tag=f"ss{s}")
                nc.scalar.activation(
                    out=e_sb[:], in_=lg_sb[:],
                    func=mybir.ActivationFunctionType.Exp,
                    bias=nmx[:], scale=1.0, accum_out=ssum[:],
                )
                rs = ps.tile([K, 1], F32, name=f"rs{s}", tag=f"rs{s}")
                nc.vector.reciprocal(out=rs[:], in_=ssum[:])
                if not nm_ones:
                    nc.vector.tensor_tensor(
                        out=rs[:], in0=rs[:], in1=d["nmcol"], op=mybir.AluOpType.mult
                    )
                find_sb = ps.tile([K, N], HALF, name=f"find{s}", tag=f"find{s}")
                nc.vector.tensor_scalar_mul(out=find_sb[:], in0=e_sb[:], scalar1=rs[:])
                d["find"] = find_sb

            def stage_proj(s):
                d = st[s]
                find_sb = d["find"]
                gfT_ps = psum.tile([N, R], F32, tag="ps", name=f"gfT_ps{s}")
                nc.tensor.matmul(
                    out=gfT_ps[:], lhsT=find_sb[:], rhs=d["GTm"], start=True, stop=True
                )
                gfT_sb = ps.tile([N, R], HALF, name=f"gfT{s}", tag=f"gfT{s}")
                nc.vector.tensor_copy(out=gfT_sb[:], in_=gfT_ps[:])
                d["gfT"] = gfT_sb
                f2T_ps = psum.tile([N, K], F32, tag="ps", name=f"f2T_ps{s}")
                nc.tensor.matmul(
                    out=f2T_ps[:], lhsT=find_sb[:], rhs=I128_sb[:K, :K],
                    start=True, stop=False,
                )
                d["f2T_ps"] = f2T_ps

            def stage_fast_f2T(s):
                # find2 scatter via the host-folded GS[kk, (r, k)] =
                # 0.5*GT[kk, r]*ST[r, k]: one matmul lands gfS, then 12
                # ea-matmuls accumulate straight into f2T — no hT transpose
                # round-trip. The sigmoid's +0.5 affine half is the
                # host-computed row t[k] broadcast via a ones outer-product.
                d = st[s]
                gfS_ps = psum.tile([N, R * K], F32, tag="ps", name=f"gfS_ps{s}")
                nc.tensor.matmul(
                    out=gfS_ps[:], lhsT=d["find"], rhs=d["GSm"],
                    start=True, stop=True,
                )
                gfS_sb = ps.tile([N, R * K], HALF, name=f"gfS{s}", tag=f"gfS{s}")
                nc.vector.tensor_copy(out=gfS_sb[:], in_=gfS_ps[:])
                f2T_ps = psum.tile([N, K], F32, tag="ps", name=f"f2T_ps{s}")
                nc.tensor.matmul(
                    out=f2T_ps[:], lhsT=d["find"], rhs=I128_sb[:K, :K],
                    start=True, stop=False,
                )
                for r in range(R):
                    nc.tensor.matmul(
                        out=f2T_ps[:], lhsT=d["ea"][:, r, :],
                        rhs=gfS_sb[:, K * r : K * (r + 1)],
                        start=False, stop=False,
                    )
                nc.tensor.matmul(
                    out=f2T_ps[:], lhsT=ones128rh_sb[:, :N], rhs=d["tm"],
                    start=False, stop=True,
                )
                d["f2T_ps"] = f2T_ps

            def stage_edge(s):
                d = st[s]
                ea_all = ps.tile([N, R, N], HALF, name=f"ea{s}", tag=f"ea{s}")
                GR = R // 2
                for g in range(2):
                    Bg = psum.tile([N, GR, N], F32, tag="ps", name=f"B6_{s}{g}")
                    for i in range(GR):
                        r = GR * g + i
                        sel = I12h_sb[:, r : r + 1].to_broadcast([K, N])
                        nc.tensor.matmul(
                            out=Bg[:, i, :], lhsT=sel, rhs=d["A0"][:],
                            start=(i == 0), stop=False,
                        )
                        nc.tensor.matmul(
                            out=Bg[:, i, :], lhsT=d["A1"][:], rhs=sel,
                            start=False, stop=(i == GR - 1),
                        )
                    # sigmoid(x) = 0.5*tanh(x/2) + 0.5; tanh shares the Exp
                    # table set, so no act-table switch mid-chain. In the
                    # fast path the affine is folded into GS/t host-side.
                    nc.scalar.activation(
                        out=ea_all[:, GR * g : GR * (g + 1), :], in_=Bg[:],
                        func=mybir.ActivationFunctionType.Tanh,
                        scale=0.5,
                    )
                if not fast:
                    nc.vector.tensor_scalar(
                        out=ea_all[:], in0=ea_all[:],
                        scalar1=0.5, scalar2=0.5,
                        op0=mybir.AluOpType.mult, op1=mybir.AluOpType.add,
                    )
                    if not rm_ones:
                        nc.vector.tensor_tensor(
                            out=ea_all[:],
                            in0=ea_all[:],
                            in1=d["rmask"][:, None, :].to_broadcast([N, R, N]),
                            op=mybir.AluOpType.mult,
                        )
                d["ea"] = ea_all

            def stage_h(s):
                d = st[s]
                h_ps = psum.tile([R, N], F32, tag="ps", name=f"h_ps{s}")
                hT_ps = psum.tile([N, R], F32, tag="ps", name=f"hT_ps{s}")
                for r in range(R):
                    nc.tensor.matmul(
                        out=hT_ps[:, r : r + 1],
                        lhsT=d["ea"][:, r, :],
                        rhs=d["gfT"][:, r : r + 1],
                        start=(r == 0),
                        stop=(r == R - 1),
                    )
                hT_sb = ps.tile([N, R], HALF, name=f"hT{s}", tag=f"hT{s}")
                nc.scalar.copy(out=hT_sb[:], in_=hT_ps[:])
                nc.tensor.matmul(
                    out=h_ps[:], lhsT=hT_sb[:], rhs=I128_sb[:N, :N],
                    start=True, stop=True,
                )
                h_sb = ps.tile([R, N], HALF, name=f"h{s}", tag=f"h{s}")
                nc.vector.tensor_copy(out=h_sb[:], in_=h_ps[:])
                nc.tensor.matmul(
                    out=d["f2T_ps"][:], lhsT=h_sb[:], rhs=d["STm"],
                    start=False, stop=True,
                )

            def stage_fa_red(s):
                # fast path, DVE part: one reduce lands the raw fa column;
                # the peak-norm runs in parallel off-path and is applied to
                # outT in the epilogue (linear, so exact). s0's fa2 column is
                # at base partition 0, so the faT matmul can read it straight
                # back; s1's block starts at partition 64 (matmul operands
                # must share a base partition), so reduce to a scratch tile
                # and let Act copy it into fa2.
                d = st[s]
                if s == 0:
                    nc.vector.tensor_reduce(
                        out=fa2_sb[:N, 0:1],
                        in_=d["f2T_ps"][:], axis=mybir.AxisListType.X,
                        op=mybir.AluOpType.max,
                    )
                    d["fa_sb"] = fa2_sb[:N, 0:1]
                else:
                    fa_sb = ps.tile([N, 1], HALF, name=f"fa{s}", tag=f"fa{s}")
                    nc.vector.tensor_reduce(
                        out=fa_sb[:], in_=d["f2T_ps"][:], axis=mybir.AxisListType.X,
                        op=mybir.AluOpType.max,
                    )
                    nc.scalar.copy(
                        out=fa2_sb[N * s : N * (s + 1), s : s + 1], in_=fa_sb[:]
                    )
                    d["fa_sb"] = fa_sb

            def stage_fa_nr(s):
                # peak over the 64 boxes = a partition-axis (C) reduce, which
                # GPSIMD can do straight off the SBUF fa column — no PE
                # transpose, no DVE involvement
                d = st[s]
                nrv = ps.tile([N, 1], F32, name=f"nrv{s}", tag=f"nrv{s}")
                nc.gpsimd.partition_all_reduce(
                    nrv[:], d["fa_sb"][:], N, bass_isa.ReduceOp.max
                )
                nr = nrv[0:1, :]
                nc.gpsimd.tensor_scalar_max(out=nr, in0=nr, scalar1=1.0)
                nc.vector.reciprocal(out=nr2row_sb[:, s : s + 1], in_=nr)

            def stage_fa_slow(s):
                d = st[s]
                # slow path (untimed): reference order is normalize, then
                # mask; do it on the row and transpose back; epilogue scale
                # is neutered (nr2 <- 1)
                fa_sb = ps.tile([N, 1], HALF, name=f"fa{s}", tag=f"fa{s}")
                nc.vector.tensor_reduce(
                    out=fa_sb[:], in_=d["f2T_ps"][:], axis=mybir.AxisListType.X,
                    op=mybir.AluOpType.max,
                )
                faT_ps = psum.tile([1, N], F32, tag="ps", name=f"faT_ps{s}")
                nc.tensor.matmul(
                    out=faT_ps[:], lhsT=fa_sb[:], rhs=I128_sb[:N, :N],
                    start=True, stop=True,
                )
                nr = ps.tile([1, 1], F32, name=f"nr{s}", tag=f"nr{s}")
                nc.vector.tensor_reduce(
                    out=nr[:], in_=faT_ps[:], axis=mybir.AxisListType.X,
                    op=mybir.AluOpType.max,
                )
                nc.vector.tensor_scalar_max(out=nr[:], in0=nr[:], scalar1=1.0)
                nc.vector.reciprocal(out=nr[:], in_=nr[:])
                nc.vector.memset(nr2row_sb[:, s : s + 1], 1.0)
                faT_sb = ps.tile([1, N], HALF, name=f"faT{s}", tag=f"faT{s}")
                nc.vector.tensor_scalar_mul(
                    out=faT_sb[:], in0=faT_ps[:], scalar1=nr[:]
                )
                nc.vector.tensor_tensor(
                    out=faT_sb[:], in0=faT_sb[:], in1=d["famul"],
                    op=mybir.AluOpType.mult,
                )
                nc.vector.tensor_tensor(
                    out=faT_sb[:], in0=faT_sb[:], in1=d["faadd"],
                    op=mybir.AluOpType.add,
                )
                fac_ps = psum.tile([N, 1], F32, tag="ps", name=f"fac_ps{s}")
                nc.tensor.matmul(
                    out=fac_ps[:], lhsT=faT_sb[:], rhs=ones1_sb[:, :1],
                    start=True, stop=True,
                )
                nc.scalar.copy(
                    out=fa2_sb[N * s : N * (s + 1), s : s + 1], in_=fac_ps[:]
                )

            # NOTE: do NOT prefetch the Sigmoid table early — the Act engine
            # holds two resident sets (sqrt+exp by now) and tanh shares the
            # exp set, so no mid-chain reloads occur.
            if fast:
                softmax2()
                for s in range(S):
                    stage_edge(s)
                for s in range(S):
                    stage_fast_f2T(s)
                    stage_fa_red(s)
                    stage_fa_nr(s)
            else:
                for s in range(S):
                    stage_softmax(s)
                for s in range(S):
                    stage_proj(s)
                for s in range(S):
                    stage_edge(s)
                for s in range(S):
                    stage_h(s)
                if bm_ones:
                    for s in range(S):
                        stage_fa_red(s)
                    for s in range(S):
                        stage_fa_nr(s)
                else:
                    for s in range(S):
                        stage_fa_slow(s)

            # w_c = visf_c^T @ fa2 = the mem chunk; split per sample so
            # sample 0's w and outT half run before sample 1's fa lands
            w_ps = psum.tile([P, len(W_CHUNKS), S], F32, tag="ps", name="w_ps")
            w_all = singles.tile([P, len(W_CHUNKS), S], HALF)

            # broadcast the peak-norm reciprocals across partitions
            nrbc_ps = psum.tile([P, S], F32, tag="ps", name="nrbc_ps")
            nc.tensor.matmul(
                out=nrbc_ps[:], lhsT=ones128r_sb[:], rhs=nr2row_sb[:],
                start=True, stop=True,
            )
            # the copy folds the fp8 W_out de-quantization (1/s_wo)
            nrbc_sb = singles.tile([P, S], F32)
            nc.vector.tensor_scalar_mul(
                out=nrbc_sb[:], in0=nrbc_ps[:], scalar1=1.0 / s_wo
            )

            # ---- tail in transposed [128, (j, s)] layout: outT[p, j, s] =
            # (sum_c sum_v WoT[v, c, 128j+p]*w[v, c, s]) * nr[s] + b[128j+p].
            # c-outer so only chunk 14/15's 8 tiny matmuls trail the last
            # WoT byte. All epilogue ops run on 128 partitions. ----
            for i in range(len(W_CHUNKS)):
                nc.tensor.matmul(
                    out=w_ps[:, i, :], lhsT=visfT_sb[i][:], rhs=fa2_sb[:],
                    start=True, stop=True,
                )
            nc.vector.tensor_copy(out=w_all[:], in_=w_ps[:])
            outT_ps = psumO.tile([P, 8, S], F32, tag="outT", name="outT_ps")
            for j in range(8):
                for i, c in enumerate(W_CHUNKS):
                    nc.tensor.matmul(
                        out=outT_ps[:, j, :],
                        lhsT=WoT_sb[:, c, P * j : P * (j + 1)],
                        rhs=w_all[:, i, :],
                        start=(i == 0), stop=(i == len(W_CHUNKS) - 1),
                    )
            outT_sb = singles.tile([P, 8, S], F32)
            nc.vector.tensor_tensor(
                out=outT_sb[:], in0=outT_ps[:],
                in1=nrbc_sb[:, None, :].to_broadcast([P, 8, S]),
                op=mybir.AluOpType.mult,
            )
            nc.vector.tensor_tensor(
                out=outT_sb[:], in0=outT_sb[:], in1=boutT_sb[:],
                op=mybir.AluOpType.add,
            )
            nc.sync.dma_start(
                out=d_out[:], in_=outT_sb[:].rearrange("p j s -> p (j s)")
            )

    nc.finalize()
    return nc


def _host_prep(inputs):
    node_rep = np.asarray(inputs["node_rep"], np.float32)
    relate_rep = np.asarray(inputs["relate_rep"], np.float32)
    relate_os = np.asarray(inputs["relate_os"])
    relate_mask = np.asarray(inputs["relate_mask"], np.float32)
    vision_feat = np.asarray(inputs["vision_feat"], np.float32)
    relation_mask = np.asarray(inputs["relation_mask"], np.float32)
    box_mask = np.asarray(inputs["box_mask"], np.float32)
    node_mask = np.asarray(inputs["node_mask"], np.float32)
    norm_w = np.asarray(inputs["norm_w"], np.float32)
    W_v = np.asarray(inputs["W_v"], np.float32)
    W_e = np.asarray(inputs["W_e"], np.float32)
    W_node = np.asarray(inputs["W_node"], np.float32)
    W_rel = np.asarray(inputs["W_rel"], np.float32)
    W_out = np.asarray(inputs["W_out"], np.float32)
    b_out = np.asarray(inputs["b_out"], np.float32)

    s_mean = np.float32(np.mean(norm_w))
    WvT = (W_v.T * s_mean).astype(np.float32)
    WnT = (W_node.T / np.float32(np.sqrt(DV))).astype(np.float32)
    WA0 = (W_rel.T @ W_e[:, :DV] / np.float32(np.sqrt(DE))).astype(np.float32)
    WA1 = (W_rel.T @ W_e[:, DV:] / np.float32(np.sqrt(DE))).astype(np.float32)
    WoT = np.ascontiguousarray(W_out.T)
    s_wv = _pow2_scale(WvT)
    s_wn = _pow2_scale(WnT)
    s_a0 = _pow2_scale(WA0)
    s_a1 = _pow2_scale(WA1)
    s_wo = _pow2_scale(WoT)
    scales = (s_wv, s_wn, s_a0, s_a1, s_wo)

    subj = relate_os[..., 1].astype(np.int64)
    obj = relate_os[..., 0].astype(np.int64)
    valid = (subj != -1).astype(np.float32)
    obj_c = np.clip(obj, 0, K - 1)
    subj_c = np.clip(subj, 0, K - 1)
    G = np.zeros((B, R, K), np.float32)
    STm = np.zeros((B, R, K), np.float32)
    bi = np.arange(B)[:, None]
    ri = np.arange(R)[None, :]
    G[bi, ri, obj_c] = valid * relate_mask
    STm[bi, ri, subj_c] = 1.0

    bmmul = (box_mask > 0).astype(np.float32)
    bmadd = (bmmul - 1.0) * np.float32(6e4)  # fp16-safe large negative
    famul = box_mask
    faadd = (1.0 - box_mask) * np.float32(1e-7)

    WvT_p = _pack(WvT * s_wv).astype(E3NP)
    WoT_p = _pack(WoT * s_wo).astype(E3NP)
    wq_p = np.concatenate(
        [_pack(WnT * s_wn), _pack(WA0 * s_a0), _pack(WA1 * s_a1)], axis=1
    ).astype(E3NP)
    I128 = np.eye(P, dtype=np.float32)
    # boutT[p, (j, s)] = b_out[128j+p], duplicated across the s positions
    bout2 = np.ascontiguousarray(
        np.repeat(b_out.reshape(8, P).T[:, :, None], S, axis=2).reshape(P, 16)
    ).astype(np.float16)

    fast = (
        bool(np.all(box_mask == 1.0))
        and bool(np.all(node_mask == 1.0))
        and bool(np.all(relation_mask == 1.0))
    )
    smf = (R * K + K) if fast else SMALLS_F

    def smalls_for(b):
        sm = np.zeros((N, smf), np.float32)
        if fast:
            # GS[kk, r*K + k] = 0.5*GT[kk, r]*ST[r, k] fuses the obj-gather,
            # the subj-scatter, and the 0.5 tanh scale into one operand; the
            # +0.5 sigmoid affine reduces to the host row t[k] (rowsum(find)
            # = node_mask = 1 in the fast path)
            GS = 0.5 * G[b].T[:, :, None] * STm[b][None, :, :]  # [K, R, K]
            sm[:K, 0 : R * K] = GS.reshape(K, R * K)
            sm[0, R * K : R * K + K] = 0.5 * G[b].sum(axis=1) @ STm[b]
            return sm
        sm[:, _SM_RM : _SM_RM + N] = relation_mask[b]
        sm[:K, _SM_BM : _SM_BM + N] = bmmul[b][None, :]
        sm[:K, _SM_BA : _SM_BA + N] = bmadd[b][None, :]
        sm[:K, _SM_GT : _SM_GT + R] = G[b].T
        sm[:R, _SM_ST : _SM_ST + K] = STm[b]
        sm[:K, _SM_NM] = node_mask[b]
        sm[0, _SM_FM : _SM_FM + N] = famul[b]
        sm[0, _SM_FA : _SM_FA + N] = faadd[b]
        return sm

    in_maps = []
    for c in range(NCORES):
        b0 = S * c
        visf2 = np.concatenate(
            [_pack(vision_feat[b]).reshape(P, 16, N) for b in range(b0, b0 + S)],
            axis=2,
        ).reshape(P, -1)
        nrep2 = np.concatenate(
            [
                _pack(np.ascontiguousarray(node_rep[b].T)).reshape(P, 4, K)
                for b in range(b0, b0 + S)
            ],
            axis=2,
        ).reshape(P, -1)
        rrep2 = np.concatenate(
            [
                _pack(np.ascontiguousarray(relate_rep[b].T)).reshape(P, 4, R)
                for b in range(b0, b0 + S)
            ],
            axis=2,
        ).reshape(P, -1)
        I12blk = np.zeros((P, K), np.float32)
        I12blk[:K, :K] = np.eye(K, dtype=np.float32)
        wcat_full = np.ascontiguousarray(
            np.concatenate([nrep2, rrep2, I12blk], axis=1)
        )
        smalls2 = np.concatenate(
            [smalls_for(b) for b in range(b0, b0 + S)], axis=1
        )  # [64, S*smf]
        rest32 = np.zeros((P, P + S * smf), np.float32)
        rest32[:, :P] = I128
        rest32[:N, P:] = smalls2
        m = {
            "visf16": np.ascontiguousarray(visf2).astype(np.float16),
            "WvT": WvT_p,
            "wcat": wcat_full.astype(np.float16),
            "wq": wq_p,
            "WoT": WoT_p,
            "bout": bout2,
            "resth": rest32.astype(np.float16),
        }
        in_maps.append(m)
    return in_maps, scales


def kernel(**inputs) -> np.ndarray:
    bm_ones = bool(np.all(np.asarray(inputs["box_mask"]) == 1.0))
    nm_ones = bool(np.all(np.asarray(inputs["node_mask"]) == 1.0))
    rm_ones = bool(np.all(np.asarray(inputs["relation_mask"]) == 1.0))
    in_maps, scales = _host_prep(inputs)
    key = ("nc", bm_ones, nm_ones, rm_ones, scales)
    if key not in _cache:
        _cache[key] = build_nc(bm_ones, nm_ones, rm_ones, scales)
    nc = _cache[key]
    res = run_bass_kernel_spmd(nc, in_maps, core_ids=list(range(NCORES)))
    outs = []
    for c in range(NCORES):
        t = np.asarray(res.results[c]["out"]).reshape(P, 8, S)
        outs.append(t.transpose(2, 1, 0).reshape(S, DC))  # out[s,128j+p]
    return np.concatenate(outs, axis=0).astype(np.float32)

